# revision 2
# baseline (speedup 1.0000x reference)
"""MoE gate kernel for Trainium2 (8 NeuronCores, SPMD).

Computes, for hidden_states [4, 4096, 2048] and gate weight [64, 2048]:
  logits = x @ W^T          (T=16384 tokens, E=64 experts)
  scores = softmax(logits)
  topk_weight, topk_idx = top_k(scores, 8), weights renormalized over the top-8
  row_idx = arange(T*K).reshape(K, T).T   (data independent)

Sharding: tokens split evenly across 8 cores (2048 tokens/core); the gate
weight is replicated.

Precision/traffic scheme (3 bytes per x element vs fp32's 4):
  x pre-scaled by 2^8, split into fp16 hi (xh) + fp8e3m4 residual
  (xl = (xs - xh) * 2^4).  W pre-scaled by 2^10, split into fp16 hi + fp16
  lo (exact residual, unscaled) packed side by side into one [H, 128]
  stationary "whl", plus an fp8e3m4 copy wc = ws * 2^-4.
  Pass AB: one fp16 matmul per h-chunk with stationary [wh|wl] (128 cols)
  and moving xh [128, 512] -> psAB [128ab, 512t]; partitions 0-63 hold
  xh*wh, 64-127 hold xh*wl.
  Pass C: fp8 matmul wc^T * xl -> psC [64, 512] which lands at exactly the
  same scale as pass A since (r*2^4)*(ws*2^-4) = r*ws.
  logits^T * 2^18 = psAB[0:64] + psAB[64:128] + psC, combined via one ACT
  copy + two DVE ops, then PE-transposed to [tokens, experts] for the
  hardware top-8 (DVE max/max_index), exp over just the top-8 (the full
  softmax normalizer cancels in the renormalization), reciprocal, scale.

Measured offline (exact simulation on the fixed inputs): rel idx err
1.5e-3, rel weight err ~4e-6 -- far inside the 2e-2 gate.

DMA: host pre-permutes each core shard to block-major [NB, 128, CH*TB]
so each 512-token block is ONE dma_start (xh 2MB fp16 on the SP HWDGE
ring, xl 1MB fp8 on the ACT ring) with 16KB/8KB of DRAM-contiguous data
per partition line -- near-peak HBM efficiency.  Weights + outputs ride
the ACT ring.  Total per-core traffic 12.7MB -> ~35.5us floor at
358 GB/s/core.
"""

import numpy as np

# -- problem constants (hardcoded per contract) --
B, S, H = 4, 4096, 2048
T = B * S                  # 16384 tokens
E = 64                     # experts
K = 8                      # top-k
N_CORES = 8
TC = T // N_CORES          # 2048 tokens per core
TB = 512                   # tokens per block (one PSUM bank of logits^T)
NB = TC // TB              # 4 blocks
P = 128                    # SBUF partitions
CH = H // P                # 16 h-chunks
NT = TB // P               # 4 token sub-tiles per block

SX = 2.0 ** 8              # x pre-scale
SW = 2.0 ** 10             # w pre-scale
S8 = 2.0 ** 4              # xl fp8 residual scale
SWC = 2.0 ** -4            # wc fp8 scale (S8*SWC == 1: psC needs no rescale)
DESCALE = 1.0 / (SX * SW)  # folded into the exp's scale argument

_CACHE = {}


def _build_program(loop_iters=1, dma_only=False):
    import concourse.bacc as bacc
    import concourse.tile as tile
    from concourse.mybir import dt, ActivationFunctionType as AFT, AluOpType
    from contextlib import ExitStack, nullcontext

    f32 = dt.float32
    f16 = dt.float16
    f8 = dt.float8e3
    u32 = dt.uint32

    nc = bacc.Bacc("TRN2", target_bir_lowering=False, debug=False,
                   num_devices=N_CORES)

    xh = nc.dram_tensor("xh", [NB, P, CH * TB], f16, kind="ExternalInput")
    xl = nc.dram_tensor("xl", [NB, P, CH * TB], f8, kind="ExternalInput")
    whl = nc.dram_tensor("whl", [P, CH * 2 * E], f16, kind="ExternalInput")
    wc = nc.dram_tensor("wc", [P, CH * E], f8, kind="ExternalInput")
    ident = nc.dram_tensor("ident", [E, E], f32, kind="ExternalInput")
    out_w = nc.dram_tensor("out_w", [TC, K], f32, kind="ExternalOutput")
    out_i = nc.dram_tensor("out_i", [TC, K], u32, kind="ExternalOutput")

    with tile.TileContext(nc) as tc:
        with ExitStack() as ctx:
            wpool = ctx.enter_context(tc.tile_pool(name="w", bufs=1))
            xhpool = ctx.enter_context(tc.tile_pool(name="xh", bufs=NB))
            xlpool = ctx.enter_context(tc.tile_pool(name="xl", bufs=NB))
            abpool = ctx.enter_context(tc.tile_pool(name="ab", bufs=2,
                                                    space="PSUM"))
            cpool = ctx.enter_context(tc.tile_pool(name="c", bufs=2,
                                                   space="PSUM"))
            tpool = ctx.enter_context(tc.tile_pool(name="t", bufs=2,
                                                   space="PSUM"))
            scpool = ctx.enter_context(tc.tile_pool(name="sc", bufs=2))
            stpool = ctx.enter_context(tc.tile_pool(name="st", bufs=2))
            smpool = ctx.enter_context(tc.tile_pool(name="sm", bufs=4))

            # weights + identity on the ACT ring, ahead of the xl stream
            whl_t = wpool.tile([P, CH * 2 * E], f16)
            nc.scalar.dma_start(whl_t[:], whl[:])
            wc_t = wpool.tile([P, CH * E], f8)
            nc.scalar.dma_start(wc_t[:], wc[:])
            id_t = wpool.tile([E, E], f32)
            nc.scalar.dma_start(id_t[:], ident[:])

            loop_cm = (tc.For_i(0, loop_iters, 1) if loop_iters > 1
                       else nullcontext())
            with loop_cm:
                # all input triggers first: HWDGE rings are FIFO per issuing
                # engine, so emitting every load before any compute keeps
                # both rings streaming continuously
                xh_b, xl_b = [], []
                for b in range(NB):
                    th = xhpool.tile([P, CH * TB], f16, tag="xh")
                    nc.sync.dma_start(th[:], xh[b])
                    tl = xlpool.tile([P, CH * TB], f8, tag="xl")
                    nc.scalar.dma_start(tl[:], xl[b])
                    xh_b.append(th)
                    xl_b.append(tl)

                if dma_only:
                    # just touch the tiles so the loop isn't empty
                    dummy = smpool.tile([P, 1], f16, tag="dummy")
                    nc.vector.tensor_scalar_mul(dummy[:], xh_b[NB - 1][:, 0:1],
                                                1.0)
                else:
                    def emit_mms(b):
                        ps_ab = abpool.tile([P, TB], f32, tag="ab")
                        for c in range(CH):
                            nc.tensor.matmul(
                                ps_ab[:], whl_t[:, c * 2 * E:(c + 1) * 2 * E],
                                xh_b[b][:, c * TB:(c + 1) * TB],
                                start=(c == 0), stop=(c == CH - 1))
                        ps_c = cpool.tile([E, TB], f32, tag="c")
                        for c in range(CH):
                            nc.tensor.matmul(
                                ps_c[:], wc_t[:, c * E:(c + 1) * E],
                                xl_b[b][:, c * TB:(c + 1) * TB],
                                start=(c == 0), stop=(c == CH - 1))
                        return ps_ab, ps_c

                    def emit_tail(b, ps_ab, ps_c):
                        # logits^T = psAB[0:64] + psAB[64:128] + psC
                        t0 = scpool.tile([E, TB], f32, tag="t0")
                        nc.scalar.copy(t0[:], ps_ab[0:E, :])
                        t1 = scpool.tile([E, TB], f32, tag="t1")
                        nc.vector.scalar_tensor_tensor(
                            t1[:], ps_ab[E:2 * E, :], 1.0, t0[:],
                            op0=AluOpType.mult, op1=AluOpType.add)
                        sc2 = scpool.tile([E, TB], f32, tag="sc2")
                        nc.vector.scalar_tensor_tensor(
                            sc2[:], ps_c[:], 1.0, t1[:],
                            op0=AluOpType.mult, op1=AluOpType.add)

                        ps_t = tpool.tile([P, NT * E], f32, tag="pt")
                        for k in range(NT):
                            nc.tensor.transpose(ps_t[:, k * E:(k + 1) * E],
                                                sc2[:, k * P:(k + 1) * P],
                                                id_t[:])
                        sc = scpool.tile([P, NT * E], f32, tag="sc")
                        nc.scalar.copy(sc[:], ps_t[:])

                        w_st = stpool.tile([P, NT, K], f32, tag="wst")
                        i_st = stpool.tile([P, NT, K], u32, tag="ist")
                        for k in range(NT):
                            sck = sc[:, k * E:(k + 1) * E]
                            mx = smpool.tile([P, K], f32, tag="mx")
                            nc.vector.max(mx[:], sck)
                            nc.vector.max_index(i_st[:, k, :], mx[:], sck)
                            ex = smpool.tile([P, K], f32, tag="ex")
                            den = smpool.tile([P, 1], f32, tag="den")
                            nc.scalar.activation(ex[:], mx[:], AFT.Exp,
                                                 scale=float(DESCALE),
                                                 accum_out=den[:])
                            rd = smpool.tile([P, 1], f32, tag="rd")
                            nc.vector.reciprocal(rd[:], den[:])
                            nc.vector.tensor_scalar_mul(w_st[:, k, :], ex[:],
                                                        rd[:, 0:1])

                        dst_w = out_w[b * TB:(b + 1) * TB, :].rearrange(
                            "(n p) k -> p n k", p=P)
                        dst_i = out_i[b * TB:(b + 1) * TB, :].rearrange(
                            "(n p) k -> p n k", p=P)
                        nc.scalar.dma_start(dst_w, w_st[:])
                        nc.scalar.dma_start(dst_i, i_st[:])

                    # software-pipelined emission: block b's MMs go to the PE
                    # queue before block b-1's transposes, so the PE never
                    # head-of-line blocks on the DVE combine of the previous
                    # block
                    prev = None
                    for b in range(NB):
                        cur = emit_mms(b)
                        if prev is not None:
                            emit_tail(b - 1, *prev)
                        prev = cur
                    emit_tail(NB - 1, *prev)

    nc.compile()
    return nc


def _get_program_loop(loop_iters):
    key = ("loop", loop_iters)
    if key not in _CACHE:
        _CACHE[key] = _build_program(loop_iters=loop_iters)
    return _CACHE[key]


def _get_program(repeats=1):
    key = ("nc", repeats)
    if key not in _CACHE:
        _CACHE[key] = _build_program()
    return _CACHE[key]


def _prepare_inputs(hidden_states, weight):
    import ml_dtypes
    e3m4 = ml_dtypes.float8_e3m4

    x = np.asarray(hidden_states, dtype=np.float32).reshape(T, H)
    w = np.asarray(weight, dtype=np.float32)

    xs = x * np.float32(SX)
    xh = xs.astype(np.float16)
    xl = ((xs - xh.astype(np.float32)) * np.float32(S8)).astype(e3m4)

    ws = w * np.float32(SW)                      # [E, H]
    wh = ws.astype(np.float16)
    wl = (ws - wh.astype(np.float32)).astype(np.float16)
    wcv = (ws * np.float32(SWC)).astype(e3m4)

    # x -> per-core block-major [NB, P, CH*TB]
    def blockify(a):
        # [T, H] -> [NC, NB, P, CH*TB]
        return np.ascontiguousarray(
            a.reshape(N_CORES, NB, TB, CH, P).transpose(0, 1, 4, 3, 2)
        ).reshape(N_CORES, NB, P, CH * TB)

    xh_b = blockify(xh)
    xl_b = blockify(xl)

    # w -> [P, CH*2E] with [wh | wl] per chunk;  wc -> [P, CH*E]
    whl = np.concatenate([wh.T, wl.T], axis=1)   # [H, 2E]
    whl_p = np.ascontiguousarray(
        whl.reshape(CH, P, 2 * E).transpose(1, 0, 2)).reshape(P, CH * 2 * E)
    wc_p = np.ascontiguousarray(
        wcv.T.reshape(CH, P, E).transpose(1, 0, 2)).reshape(P, CH * E)
    ident = np.eye(E, dtype=np.float32)

    return [
        {"xh": xh_b[i], "xl": xl_b[i], "whl": whl_p, "wc": wc_p,
         "ident": ident}
        for i in range(N_CORES)
    ]


def _enable_jax_compile_cache():
    try:
        import os
        import jax
        jax.config.update("jax_compilation_cache_dir",
                          os.path.expanduser("~/.cache/jax_bass_cache"))
        jax.config.update("jax_persistent_cache_min_entry_size_bytes", -1)
        jax.config.update("jax_persistent_cache_min_compile_time_secs", 0)
    except Exception:
        pass


def kernel(hidden_states, weight):
    from concourse.bass_utils import run_bass_kernel_spmd

    _enable_jax_compile_cache()
    in_maps = _prepare_inputs(hidden_states, weight)
    nc = _get_program()
    res = run_bass_kernel_spmd(nc, in_maps, list(range(N_CORES))).results

    topk_w = np.concatenate([res[i]["out_w"] for i in range(N_CORES)], axis=0)
    topk_i = np.concatenate([res[i]["out_i"] for i in range(N_CORES)],
                            axis=0).astype(np.int32)
    row_idx = np.arange(T * K, dtype=np.int32).reshape(K, T).T
    return topk_i, topk_w.astype(np.float32), row_idx


# revision 8
# speedup vs baseline: 1.2175x; 1.2175x over previous
"""MoE gate kernel for Trainium2 (8 NeuronCores, SPMD).

Computes, for hidden_states [4, 4096, 2048] and gate weight [64, 2048]:
  logits = x @ W^T          (T=16384 tokens, E=64 experts)
  scores = softmax(logits)
  topk_weight, topk_idx = top_k(scores, 8), weights renormalized over the top-8
  row_idx = arange(T*K).reshape(K, T).T   (data independent)

Sharding: tokens split evenly across 8 cores (2048 tokens/core); the gate
weight is replicated.

Precision/traffic scheme (3 bytes per x element vs fp32's 4):
  x pre-scaled by 2^8, split into fp16 hi (xh) + fp8e3m4 residual
  (xl = (xs - xh) * 2^4).  W pre-scaled by 2^10, split into fp16 hi + fp16
  lo (exact residual, unscaled) packed side by side into one [H, 128]
  stationary "whl", plus an fp8e3m4 copy wc = ws * 2^-4.
  Pass AB: one fp16 matmul per h-chunk with stationary [wh|wl] (128 cols)
  and moving xh [128, 512] -> psAB [128ab, 512t]; partitions 0-63 hold
  xh*wh, 64-127 hold xh*wl.
  Pass C: fp8 matmul wc^T * xl -> psC [64, 512] which lands at exactly the
  same scale as pass A since (r*2^4)*(ws*2^-4) = r*ws.
  logits^T * 2^18 = psAB[0:64] + psAB[64:128] + psC, combined via one ACT
  copy + two DVE ops, then PE-transposed to [tokens, experts] for the
  hardware top-8 (DVE max/max_index), exp over just the top-8 (the full
  softmax normalizer cancels in the renormalization), reciprocal, scale.

Measured offline (exact simulation on the fixed inputs): rel idx err
1.5e-3, rel weight err ~4e-6 -- far inside the 2e-2 gate.

DMA: host pre-permutes each core shard to block-major [NB, 128, CH*TB]
so each 512-token block is ONE dma_start (xh 2MB fp16 on the SP HWDGE
ring, xl 1MB fp8 on the ACT ring) with 16KB/8KB of DRAM-contiguous data
per partition line -- near-peak HBM efficiency.  Weights + outputs ride
the ACT ring.  Total per-core traffic 12.7MB -> ~35.5us floor at
358 GB/s/core.
"""

import numpy as np

# -- problem constants (hardcoded per contract) --
B, S, H = 4, 4096, 2048
T = B * S                  # 16384 tokens
E = 64                     # experts
K = 8                      # top-k
N_CORES = 8
TC = T // N_CORES          # 2048 tokens per core
TB = 512                   # tokens per block (one PSUM bank of logits^T)
NB = TC // TB              # 4 blocks
P = 128                    # SBUF partitions
CH = H // P                # 16 h-chunks
NT = TB // P               # 4 token sub-tiles per block

SX = 2.0 ** 8              # x pre-scale
SW = 2.0 ** 10             # w pre-scale
S8 = 2.0 ** 4              # xl fp8 residual scale
SWC = 2.0 ** -4            # wc fp8 scale (S8*SWC == 1: psC needs no rescale)
DESCALE = 1.0 / (SX * SW)  # folded into the exp's scale argument

_CACHE = {}


def _build_program(loop_iters=1, variant="full"):
    dma_only = variant in ("dma", "dma1")
    one_ring = variant == "dma1"
    compute_only = variant == "compute"
    import concourse.bacc as bacc
    import concourse.tile as tile
    from concourse.mybir import dt, ActivationFunctionType as AFT, AluOpType
    from contextlib import ExitStack, nullcontext

    f32 = dt.float32
    f16 = dt.float16
    f8 = dt.float8e3
    u32 = dt.uint32

    nc = bacc.Bacc("TRN2", target_bir_lowering=False, debug=False,
                   num_devices=N_CORES)

    xh = nc.dram_tensor("xh", [NB, P, CH * TB], f16, kind="ExternalInput")
    xl = nc.dram_tensor("xl", [NB, P, CH * TB], f8, kind="ExternalInput")
    whl = nc.dram_tensor("whl", [P, CH * 2 * E], f16, kind="ExternalInput")
    wc = nc.dram_tensor("wc", [P, CH * E], f8, kind="ExternalInput")
    ident = nc.dram_tensor("ident", [E, E], f32, kind="ExternalInput")
    out_w = nc.dram_tensor("out_w", [TC, K], f32, kind="ExternalOutput")
    out_i = nc.dram_tensor("out_i", [TC, K], u32, kind="ExternalOutput")

    with tile.TileContext(nc) as tc:
        with ExitStack() as ctx:
            wpool = ctx.enter_context(tc.tile_pool(name="w", bufs=1))
            xhpool = ctx.enter_context(tc.tile_pool(name="xh", bufs=NB))
            xlpool = ctx.enter_context(tc.tile_pool(name="xl", bufs=NB))
            abpool = ctx.enter_context(tc.tile_pool(name="ab", bufs=2,
                                                    space="PSUM"))
            cpool = ctx.enter_context(tc.tile_pool(name="c", bufs=2,
                                                   space="PSUM"))
            tpool = ctx.enter_context(tc.tile_pool(name="t", bufs=2,
                                                   space="PSUM"))
            scpool = ctx.enter_context(tc.tile_pool(name="sc", bufs=2))
            stpool = ctx.enter_context(tc.tile_pool(name="st", bufs=2))
            smpool = ctx.enter_context(tc.tile_pool(name="sm", bufs=4))

            # weights + identity on the ACT ring, ahead of the xl stream
            whl_t = wpool.tile([P, CH * 2 * E], f16)
            nc.scalar.dma_start(whl_t[:], whl[:])
            wc_t = wpool.tile([P, CH * E], f8)
            nc.scalar.dma_start(wc_t[:], wc[:])
            id_t = wpool.tile([E, E], f32)
            nc.scalar.dma_start(id_t[:], ident[:])

            loop_cm = (tc.For_i(0, loop_iters, 1) if loop_iters > 1
                       else nullcontext())

            if compute_only:
                # load all inputs once, outside the loop
                xh_b, xl_b = [], []
                for b in range(NB):
                    th = xhpool.tile([P, CH * TB], f16, tag="xh")
                    nc.sync.dma_start(th[:], xh[b])
                    tl = xlpool.tile([P, CH * TB], f8, tag="xl")
                    nc.scalar.dma_start(tl[:], xl[b])
                    xh_b.append(th)
                    xl_b.append(tl)

            with loop_cm:
                # all input triggers first: HWDGE rings are FIFO per issuing
                # engine, so emitting every load before any compute keeps
                # both rings streaming continuously
                if not compute_only:
                    # ALL input loads ride the SP HWDGE ring: SP issues
                    # nothing else, so input streaming can never head-of-line
                    # block behind compute (the ACT ring's triggers queue
                    # after the previous iteration's copies/exp otherwise,
                    # serializing DMA with compute across loop iterations)
                    xh_b, xl_b = [], []
                    for b in range(NB):
                        th = xhpool.tile([P, CH * TB], f16, tag="xh")
                        nc.sync.dma_start(th[:], xh[b])
                        tl = xlpool.tile([P, CH * TB], f8, tag="xl")
                        if one_ring or not dma_only:
                            nc.sync.dma_start(tl[:], xl[b])
                        else:
                            nc.scalar.dma_start(tl[:], xl[b])
                        xh_b.append(th)
                        xl_b.append(tl)

                if dma_only:
                    # just touch the tiles so the loop isn't empty
                    dummy = smpool.tile([P, 1], f16, tag="dummy")
                    nc.vector.tensor_scalar_mul(dummy[:], xh_b[NB - 1][:, 0:1],
                                                1.0)
                else:
                    def emit_mms(b):
                        ps_ab = abpool.tile([P, TB], f32, tag="ab")
                        for c in range(CH):
                            nc.tensor.matmul(
                                ps_ab[:], whl_t[:, c * 2 * E:(c + 1) * 2 * E],
                                xh_b[b][:, c * TB:(c + 1) * TB],
                                start=(c == 0), stop=(c == CH - 1))
                        ps_c = cpool.tile([E, TB], f32, tag="c")
                        for c in range(CH):
                            nc.tensor.matmul(
                                ps_c[:], wc_t[:, c * E:(c + 1) * E],
                                xl_b[b][:, c * TB:(c + 1) * TB],
                                start=(c == 0), stop=(c == CH - 1))
                        return ps_ab, ps_c

                    def emit_tail(b, ps_ab, ps_c):
                        # logits^T = psAB[0:64] + psAB[64:128] + psC
                        t0 = scpool.tile([E, TB], f32, tag="t0")
                        nc.scalar.copy(t0[:], ps_ab[0:E, :])
                        t1 = scpool.tile([E, TB], f32, tag="t1")
                        nc.vector.scalar_tensor_tensor(
                            t1[:], ps_ab[E:2 * E, :], 1.0, t0[:],
                            op0=AluOpType.mult, op1=AluOpType.add)
                        sc2 = scpool.tile([E, TB], f32, tag="sc2")
                        nc.vector.scalar_tensor_tensor(
                            sc2[:], ps_c[:], 1.0, t1[:],
                            op0=AluOpType.mult, op1=AluOpType.add)

                        ps_t = tpool.tile([P, NT * E], f32, tag="pt")
                        for k in range(NT):
                            nc.tensor.transpose(ps_t[:, k * E:(k + 1) * E],
                                                sc2[:, k * P:(k + 1) * P],
                                                id_t[:])
                        sc = scpool.tile([P, NT * E], f32, tag="sc")
                        nc.scalar.copy(sc[:], ps_t[:])

                        w_st = stpool.tile([P, NT, K], f32, tag="wst")
                        i_st = stpool.tile([P, NT, K], u32, tag="ist")
                        for k in range(NT):
                            sck = sc[:, k * E:(k + 1) * E]
                            mx = smpool.tile([P, K], f32, tag="mx")
                            nc.vector.max(mx[:], sck)
                            nc.vector.max_index(i_st[:, k, :], mx[:], sck)
                            ex = smpool.tile([P, K], f32, tag="ex")
                            den = smpool.tile([P, 1], f32, tag="den")
                            nc.scalar.activation(ex[:], mx[:], AFT.Exp,
                                                 scale=float(DESCALE),
                                                 accum_out=den[:])
                            rd = smpool.tile([P, 1], f32, tag="rd")
                            nc.vector.reciprocal(rd[:], den[:])
                            nc.vector.tensor_scalar_mul(w_st[:, k, :], ex[:],
                                                        rd[:, 0:1])

                        dst_w = out_w[b * TB:(b + 1) * TB, :].rearrange(
                            "(n p) k -> p n k", p=P)
                        dst_i = out_i[b * TB:(b + 1) * TB, :].rearrange(
                            "(n p) k -> p n k", p=P)
                        nc.scalar.dma_start(dst_w, w_st[:])
                        nc.scalar.dma_start(dst_i, i_st[:])

                    # software-pipelined emission: block b's MMs go to the PE
                    # queue before block b-1's transposes, so the PE never
                    # head-of-line blocks on the DVE combine of the previous
                    # block
                    prev = None
                    for b in range(NB):
                        cur = emit_mms(b)
                        if prev is not None:
                            emit_tail(b - 1, *prev)
                        prev = cur
                    emit_tail(NB - 1, *prev)

    nc.compile()
    return nc


def _get_program_loop(loop_iters, variant="full"):
    key = ("loop", loop_iters, variant)
    if key not in _CACHE:
        _CACHE[key] = _build_program(loop_iters=loop_iters, variant=variant)
    return _CACHE[key]


def _get_program(repeats=1):
    key = ("nc", repeats)
    if key not in _CACHE:
        _CACHE[key] = _build_program()
    return _CACHE[key]


def _prepare_inputs(hidden_states, weight):
    import ml_dtypes
    e3m4 = ml_dtypes.float8_e3m4

    x = np.asarray(hidden_states, dtype=np.float32).reshape(T, H)
    w = np.asarray(weight, dtype=np.float32)

    xs = x * np.float32(SX)
    xh = xs.astype(np.float16)
    xl = ((xs - xh.astype(np.float32)) * np.float32(S8)).astype(e3m4)

    ws = w * np.float32(SW)                      # [E, H]
    wh = ws.astype(np.float16)
    wl = (ws - wh.astype(np.float32)).astype(np.float16)
    wcv = (ws * np.float32(SWC)).astype(e3m4)

    # x -> per-core block-major [NB, P, CH*TB]
    def blockify(a):
        # [T, H] -> [NC, NB, P, CH*TB]
        return np.ascontiguousarray(
            a.reshape(N_CORES, NB, TB, CH, P).transpose(0, 1, 4, 3, 2)
        ).reshape(N_CORES, NB, P, CH * TB)

    xh_b = blockify(xh)
    xl_b = blockify(xl)

    # w -> [P, CH*2E] with [wh | wl] per chunk;  wc -> [P, CH*E]
    whl = np.concatenate([wh.T, wl.T], axis=1)   # [H, 2E]
    whl_p = np.ascontiguousarray(
        whl.reshape(CH, P, 2 * E).transpose(1, 0, 2)).reshape(P, CH * 2 * E)
    wc_p = np.ascontiguousarray(
        wcv.T.reshape(CH, P, E).transpose(1, 0, 2)).reshape(P, CH * E)
    ident = np.eye(E, dtype=np.float32)

    return [
        {"xh": xh_b[i], "xl": xl_b[i], "whl": whl_p, "wc": wc_p,
         "ident": ident}
        for i in range(N_CORES)
    ]


def _enable_jax_compile_cache():
    try:
        import os
        import jax
        jax.config.update("jax_compilation_cache_dir",
                          os.path.expanduser("~/.cache/jax_bass_cache"))
        jax.config.update("jax_persistent_cache_min_entry_size_bytes", -1)
        jax.config.update("jax_persistent_cache_min_compile_time_secs", 0)
    except Exception:
        pass


def kernel(hidden_states, weight):
    from concourse.bass_utils import run_bass_kernel_spmd

    _enable_jax_compile_cache()
    in_maps = _prepare_inputs(hidden_states, weight)
    nc = _get_program()
    res = run_bass_kernel_spmd(nc, in_maps, list(range(N_CORES))).results

    topk_w = np.concatenate([res[i]["out_w"] for i in range(N_CORES)], axis=0)
    topk_i = np.concatenate([res[i]["out_i"] for i in range(N_CORES)],
                            axis=0).astype(np.int32)
    row_idx = np.arange(T * K, dtype=np.int32).reshape(K, T).T
    return topk_i, topk_w.astype(np.float32), row_idx
